# revision 1
# baseline (speedup 1.0000x reference)
"""Trainium2 Bass/Tile kernel for nn_ChannelMerger.

Reference computation (per batch b):
    emb[c, d]   = fourier_embedding(positions[c])          # d = 2048
    scores[o,c] = sum_d emb[c,d] * heads[o,d] + offset[c]
    w[o,c]      = softmax_c(scores)
    out[o,t]    = sum_c x[c,t] * w[o,c]

Shapes: B=64, C=273, T=2048, O=256, D=2048 (n_freqs=32).
Sharding: data-parallel over B across 8 cores (8 batches per core).
The bc axis is laid out in 274-wide per-batch segments (fp32r matmuls
need an even moving-dim; the pad column carries zeros end to end).

Device algorithm (per core):
  * turns-domain fourier embedding, f[ij, bc] = i*u[bc] + j*v[bc] with
    u = (posx+margin)/width, v likewise:
      - u, v are encoded host-side into 3 bf16 limbs each (a lossless
        precision split of the 4K rescaled position scalars), so a K=6
        bf16 matmul against the exact integer rows [i,i,i,j,j,j]
        reproduces f at ~fp32 precision at full PE rate (products are
        exact; PSUM accumulates in fp32).
      - a parallel K=7 matmul appends a constant row M = 1.5*2^23 as the
        LAST contraction row, so PSUM's round-to-nearest of (f + M)
        yields f' = M + round(f) exactly; DVE: k = f' - M, rs = f - k in
        [-0.5, 0.5] (both exact; the DVE mod ALU op is not valid ISA).
      - sin(2*pi*f) = Sin(2*pi*rs); cos(2*pi*f) = Sin(pi/2 - 2*pi*|rs|)
        with |rs| via ACT Abs (in every table set, no switch).
  * scores: fp32r matmuls, heads pre-transposed on host to [D, O];
    invalid-mask offsets are added via a K=1 ones-matmul accumulate;
    PSUM eviction to SBUF on DVE (Pool cannot access PSUM on TRN2).
  * softmax: Exp with accum_out gives the row sums for free; reciprocal
    on DVE; the 1/sum scaling is folded into the mix-output eviction.
  * mix: per-b transpose of the exp'd score block via PE transpose
    (c-chunks {128,128,17}), bf16 matmuls against x (x is cast to bf16
    on host - halves the input DMA), PSUM eviction fused with softmax
    normalization and bf16 output cast (spread over DVE/ACT), out-DMAs
    in bf16 (halves output DMA) alternating between the two HWDGE
    queues; the host upcasts to f32.
  * ACT table switches (Sin <-> Exp) are limited to 2 per half-problem
    by explicit ordering edges between the sin and exp instruction
    groups, letting batches 0-3 flow through softmax+mix while the
    second half's embedding work is still running.
"""

import math
import time

import ml_dtypes
import numpy as np

import concourse.bacc as bacc
import concourse.tile as tile
from concourse import mybir
from concourse.tile import add_dep_helper

F32 = mybir.dt.float32
F32R = mybir.dt.float32r
BF16 = mybir.dt.bfloat16

B, C, T, O, D = 64, 273, 2048, 256, 2048
NF = 32
NIJ = NF * NF
NCORES = 8
BLOC = B // NCORES
BC = BLOC * C        # 2184
BCPAD = 2304         # 128*18 padded wrap layout for position prep
MARGIN = 0.2
WIDTH = 1.0 + 2.0 * MARGIN

SEG = 274            # padded per-batch segment width (fp32r needs even N)
BCL = BLOC * SEG     # 2192 padded columns
QWL = 2 * SEG        # 548 (quarter = 2 batches, padded)
PI = math.pi

_CACHE = {}
LAST_RUN_NS = None


def _consts():
    p = np.arange(NIJ)
    i = (p // NF).astype(np.float32)
    j = (p % NF).astype(np.float32)
    ones_row = np.ones_like(i)
    f6 = np.stack([i, i, i, j, j, j, ones_row]).astype(ml_dtypes.bfloat16)
    ident = np.eye(128, dtype=np.float32)
    ones1 = np.ones((1, 128), dtype=ml_dtypes.bfloat16)
    return f6, ident, ones1


def build(nc=None):
    nc = nc or bacc.Bacc("TRN2", target_bir_lowering=False, debug=False,
                         enable_partition_id=False)

    x_in = nc.dram_tensor("x", [BLOC, C, T], BF16, kind="ExternalInput")
    u_in = nc.dram_tensor("u", [7, BCPAD], BF16, kind="ExternalInput")
    offs_in = nc.dram_tensor("offs", [1, BCL], BF16, kind="ExternalInput")
    headsT_in = nc.dram_tensor("headsT", [D, O], F32R, kind="ExternalInput")
    out_dram = nc.dram_tensor("out", [BLOC, O, T], BF16, kind="ExternalOutput")

    f6_np, ident_np, ones_np = _consts()
    f6_dram = nc.inline_tensor(f6_np, "f6c")
    ident_dram = nc.inline_tensor(ident_np, "identc")
    ones_dram = nc.inline_tensor(ones_np, "onesc")

    with tile.TileContext(nc) as tc:
        _build_tile(tc, x_in, u_in, offs_in, headsT_in, out_dram,
                    f6_dram, ident_dram, ones_dram)
    nc.compile()
    return nc


def _build_tile(tc, x_in, u_in, offs_in, headsT_in, out_dram,
                f6_dram, ident_dram, ones_dram):
    nc = tc.nc
    Sin = mybir.ActivationFunctionType.Sin
    Exp = mybir.ActivationFunctionType.Exp
    ALU = mybir.AluOpType

    import contextlib
    ctx = contextlib.ExitStack()

    singles = ctx.enter_context(tc.tile_pool(name="singles", bufs=1))
    # U[6, BCPAD]: 3 bf16 limbs of u=(posx+m)/w, 3 of v; host-encoded so
    # the embedding matmuls can start as soon as this one DMA lands.
    # u/f6 ride the scalar-engine HWDGE queue so the first embedding
    # matmuls don't wait behind the 2MB heads DMA on the sync queue.
    u_sb = singles.tile([7, BCPAD], BF16, name="u_sb")
    nc.scalar.dma_start(out=u_sb, in_=u_in.ap())
    f6_sb = singles.tile([7, NIJ], BF16, name="f6_sb")
    nc.scalar.dma_start(out=f6_sb, in_=f6_dram.ap())
    ident_sb = singles.tile([128, 128], F32R, name="ident_sb")
    nc.sync.dma_start(out=ident_sb, in_=ident_dram.ap().bitcast(F32R))
    ones_sb = singles.tile([1, 128], BF16, name="ones_sb")
    nc.sync.dma_start(out=ones_sb, in_=ones_dram.ap())
    offs_sb = singles.tile([1, BCL], BF16, name="offs_sb")
    nc.sync.dma_start(out=offs_sb, in_=offs_in.ap())
    hpi_sb = singles.tile([128, 1], F32, name="hpi_sb")
    nc.vector.memset(hpi_sb, PI / 2)

    # heads, pre-transposed on host: hT[dl, ic*O + o] = headsT[ic*128+dl, o]
    hT = singles.tile([128, 16 * O], F32R, name="hT")
    nc.sync.dma_start(
        out=hT.rearrange("dl (ic o) -> dl ic o", o=O),
        in_=headsT_in.ap().rearrange("(ic dl) o -> dl ic o", dl=128))

    # --- pools ---
    # PSUM budget (8 banks): f 3 + scores 1 + transpose 1 + mix 3
    f_ps = ctx.enter_context(tc.tile_pool(name="f_ps", bufs=2, space="PSUM"))
    sc_ps = ctx.enter_context(tc.tile_pool(name="sc_ps", bufs=1, space="PSUM"))
    tp_ps = ctx.enter_context(tc.tile_pool(name="tp_ps", bufs=1, space="PSUM"))
    mix_ps = ctx.enter_context(tc.tile_pool(name="mix_ps", bufs=2, space="PSUM"))

    rs_pool = ctx.enter_context(tc.tile_pool(name="rs_pool", bufs=4))
    k_pool = ctx.enter_context(tc.tile_pool(name="k_pool", bufs=4))
    trig_pool = ctx.enter_context(tc.tile_pool(name="trig_pool", bufs=6))
    sc_sb_pool = ctx.enter_context(tc.tile_pool(name="sc_sb", bufs=1))
    sums_pool = ctx.enter_context(tc.tile_pool(name="sums", bufs=1))
    wt_pool = ctx.enter_context(tc.tile_pool(name="wt", bufs=4))
    x_pool = ctx.enter_context(tc.tile_pool(name="x_pool", bufs=2))
    oev_pool = ctx.enter_context(tc.tile_pool(name="oev", bufs=2))

    SC = [sc_sb_pool.tile([128, BCL], F32R, name=f"SC{oc}") for oc in range(2)]
    sums = sums_pool.tile([128, 2 * BLOC], F32, name="sums")
    rsums = sums_pool.tile([128, 2 * BLOC], F32, name="rsums")

    CW = [(0, 128), (128, 128), (256, C - 256)]

    sin_insts_group = [[], []]
    exp_insts_group = [[], []]
    trig_list = {}

    for g in range(2):
        # ---------- embedding + scores (2 quarters per group) ----------
        for qq in range(2):
            q = g * 2 + qq
            for pc in range(4):      # paired ij-chunks
                # rs tile: [rs block | abs block], each 2*QWL
                rs_t = rs_pool.tile([128, 4 * QWL], F32, tag="rs")
                for half in range(2):
                    ic = 2 * pc + half
                    for bi in range(2):
                        ucol = q * QWL + bi * SEG
                        fp = f_ps.tile([128, SEG], F32, tag="f", name="fp")
                        nc.tensor.matmul(
                            fp,
                            f6_sb[:6, ic * 128:(ic + 1) * 128],
                            u_sb[:6, ucol:ucol + SEG],
                            start=True, stop=True)
                        # f' = f + M in one K=7 matmul; the constant row
                        # rides last so the exact round-to-nearest happens
                        # after the data sum
                        fp2 = f_ps.tile([128, SEG], F32, tag="f2", name="fp2")
                        nc.tensor.matmul(
                            fp2,
                            f6_sb[:, ic * 128:(ic + 1) * 128],
                            u_sb[:, ucol:ucol + SEG],
                            start=True, stop=True, skip_group_check=True)
                        col = half * QWL + bi * SEG
                        kt = k_pool.tile([128, SEG], F32, tag="kt", name="kt")
                        nc.vector.tensor_scalar(
                            kt, fp2, 12582912.0, None, ALU.subtract)
                        # rs = f - round(f)  in [-0.5, 0.5], exact
                        nc.vector.scalar_tensor_tensor(
                            rs_t[:, col:col + SEG], fp, 0.0, kt,
                            ALU.add, ALU.subtract)
                # |rs| via ACT Abs (TensorScalar is not valid ISA on Pool;
                # Abs lives in every ACT table set, so no table switch)
                nc.scalar.activation(rs_t[:, 2 * QWL:], rs_t[:, :2 * QWL],
                                     mybir.ActivationFunctionType.Abs)
                trig_t = trig_pool.tile([128, 4 * QWL], F32R, tag="trig",
                                        name=f"trig_q{q}p{pc}")
                si = nc.scalar.activation(trig_t[:, :2 * QWL],
                                          rs_t[:, :2 * QWL], Sin,
                                          bias=0.0, scale=2 * PI)
                ci = nc.scalar.activation(trig_t[:, 2 * QWL:],
                                          rs_t[:, 2 * QWL:], Sin,
                                          bias=hpi_sb, scale=-2 * PI)
                sin_insts_group[g] += [si, ci]
                if g == 1:
                    for e in exp_insts_group[0]:
                        add_dep_helper(si.ins, e.ins, sync=False,
                                       reason="ACT order: g1 sins after g0 exps")
                        add_dep_helper(ci.ins, e.ins, sync=False,
                                       reason="ACT order: g1 sins after g0 exps")
                trig_list[(q, pc)] = trig_t

            for oc in range(2):
                for bi in range(2):
                    b = q * 2 + bi
                    sp = sc_ps.tile([128, SEG], F32, tag="sc", name="sp")
                    first = True
                    for pc in range(4):
                        trig_t = trig_list[(q, pc)]
                        for half in range(2):
                            ic = 2 * pc + half
                            col = half * QWL + bi * SEG
                            # cos block (cols 2*QWL..), then sin block
                            nc.tensor.matmul(
                                sp,
                                hT[:, ic * O + oc * 128: ic * O + oc * 128 + 128],
                                trig_t[:, 2 * QWL + col:2 * QWL + col + SEG],
                                start=first, stop=False, skip_group_check=True)
                            first = False
                            nc.tensor.matmul(
                                sp,
                                hT[:, (8 + ic) * O + oc * 128: (8 + ic) * O + oc * 128 + 128],
                                trig_t[:, col:col + SEG],
                                start=False, stop=False, skip_group_check=True)
                    nc.tensor.matmul(
                        sp, ones_sb,
                        offs_sb[:, b * SEG:b * SEG + SEG],
                        start=False, stop=True, skip_group_check=True)
                    nc.vector.tensor_copy(SC[oc][:, b * SEG:b * SEG + SEG],
                                          sp.bitcast(F32R))

        # ---------- softmax + mix (4 batches per group) ----------
        for bi in range(4):
            b = g * 4 + bi
            seg = slice(b * SEG, b * SEG + C)
            for oc in range(2):
                ei = nc.scalar.activation(
                    SC[oc][:, seg], SC[oc][:, seg], Exp,
                    accum_out=sums[:, oc * BLOC + b: oc * BLOC + b + 1])
                for s in sin_insts_group[g]:
                    add_dep_helper(ei.ins, s.ins, sync=False,
                                   reason="ACT order: exps after group sins")
                exp_insts_group[g].append(ei)
                nc.vector.reciprocal(
                    rsums[:, oc * BLOC + b: oc * BLOC + b + 1],
                    sums[:, oc * BLOC + b: oc * BLOC + b + 1])

            wts = []
            for kc, (c0, cw) in enumerate(CW):
                wt = wt_pool.tile([128, O], BF16, tag=f"wt{kc}")
                wts.append(wt)
                tp = tp_ps.tile([128, O], F32R, tag="tp", name="tp")
                for oc in range(2):
                    nc.tensor.transpose(
                        tp[:cw, oc * 128:(oc + 1) * 128],
                        SC[oc][:, b * SEG + c0: b * SEG + c0 + cw],
                        ident_sb)
                nc.vector.tensor_copy(wt[:cw, :], tp[:cw, :])

            xts = []
            for kc, (c0, cw) in enumerate(CW):
                xt = x_pool.tile([128, T], BF16, tag=f"x{kc}")
                xts.append(xt)
                nc.sync.dma_start(out=xt[:cw, :], in_=x_in.ap()[b, c0:c0 + cw, :])

            for oc in range(2):
                oe = oev_pool.tile([128, T], BF16, tag="oe")
                rsum_col = rsums[:, oc * BLOC + b: oc * BLOC + b + 1]
                for tt in range(4):
                    op = mix_ps.tile([128, 512], F32, tag="mo", name="mop")
                    for kc, (c0, cw) in enumerate(CW):
                        nc.tensor.matmul(
                            op,
                            wts[kc][:cw, oc * 128:(oc + 1) * 128],
                            xts[kc][:cw, tt * 512:(tt + 1) * 512],
                            start=(kc == 0), stop=(kc == 2),
                            skip_group_check=True)
                    oslice = oe[:, tt * 512:(tt + 1) * 512]
                    # spread evictions DVE/ACT (Pool cannot read PSUM)
                    if tt % 2 == 1:
                        nc.scalar.activation(
                            oslice, op, mybir.ActivationFunctionType.Copy,
                            bias=0.0, scale=rsum_col)
                    else:
                        nc.vector.tensor_scalar(oslice, op, rsum_col,
                                                None, ALU.mult)
                # two half-DMAs so the first overlaps the tail evictions
                out_eng = nc.scalar if b % 2 == 0 else nc.sync
                for hh in range(2):
                    out_eng.dma_start(
                        out=out_dram.ap()[b, oc * 128:(oc + 1) * 128,
                                          hh * 1024:(hh + 1) * 1024],
                        in_=oe[:, hh * 1024:(hh + 1) * 1024])

    ctx.close()


# --------------------------------------------------------------------------
# host side
# --------------------------------------------------------------------------

def _host_inputs(x, positions, invalid_mask, heads):
    headsT = np.ascontiguousarray(np.asarray(heads, dtype=np.float32).T)
    x_bf = np.asarray(x, dtype=np.float32).astype(ml_dtypes.bfloat16)
    in_maps = []
    for core in range(NCORES):
        bsl = slice(core * BLOC, (core + 1) * BLOC)
        xs = np.ascontiguousarray(x_bf[bsl])

        pos = positions[bsl].reshape(BLOC, C, 2).astype(np.float64)
        uvw = np.zeros((2, BCPAD), dtype=np.float64)
        for ax in range(2):
            seg = uvw[ax, :BCL].reshape(BLOC, SEG)
            seg[:, :C] = (pos[:, :, ax] + MARGIN) / WIDTH
        u6 = np.zeros((7, BCPAD), dtype=ml_dtypes.bfloat16)
        for ax in range(2):
            resid = uvw[ax].copy()
            for li in range(3):
                limb = resid.astype(ml_dtypes.bfloat16)
                u6[3 * ax + li] = limb
                resid = resid - limb.astype(np.float64)
        u6[6] = ml_dtypes.bfloat16(12582912.0)

        offs = np.zeros((1, BCL), dtype=np.float32)
        offs[0, :].reshape(BLOC, SEG)[:, :C] = np.where(
            invalid_mask[bsl], -1e30, 0.0)
        offs = offs.astype(ml_dtypes.bfloat16)

        in_maps.append({"x": xs, "u": u6, "offs": offs, "headsT": headsT})
    return in_maps


def kernel(**inputs):
    global LAST_RUN_NS
    from concourse.bass_utils import run_bass_kernel_spmd

    x = np.asarray(inputs["x"])
    positions = np.asarray(inputs["positions"])
    invalid_mask = np.asarray(inputs["invalid_mask"])
    heads = np.asarray(inputs["heads"])

    if "nc" not in _CACHE:
        _CACHE["nc"] = build()
    nc = _CACHE["nc"]

    in_maps = _host_inputs(x, positions, invalid_mask, heads)
    t0 = time.perf_counter()
    res = run_bass_kernel_spmd(nc, in_maps, core_ids=list(range(NCORES)))
    LAST_RUN_NS = (time.perf_counter() - t0) * 1e9
    out = np.concatenate([r["out"] for r in res.results], axis=0)
    return out.astype(np.float32)



# revision 27
# speedup vs baseline: 322.8681x; 322.8681x over previous
"""Trainium2 Bass/Tile kernel for nn_ChannelMerger.

Reference computation (per batch b):
    emb[c, d]   = fourier_embedding(positions[c])          # d = 2048
    scores[o,c] = sum_d emb[c,d] * heads[o,d] + offset[c]
    w[o,c]      = softmax_c(scores)
    out[o,t]    = sum_c x[c,t] * w[o,c]

Shapes: B=64, C=273, T=2048, O=256, D=2048 (n_freqs=32).
Sharding: data-parallel over B across 8 cores (8 batches per core).
The bc axis is laid out in 274-wide per-batch segments (fp32r matmuls
need an even moving-dim; the pad column carries zeros end to end).

Device algorithm (per core):
  * turns-domain fourier embedding, f[ij, bc] = i*u[bc] + j*v[bc] with
    u = (posx+margin)/width, v likewise:
      - u, v are encoded host-side into 3 bf16 limbs each (a lossless
        precision split of the 4K rescaled position scalars), so a K=6
        bf16 matmul against the exact integer rows [i,i,i,j,j,j]
        reproduces f at ~fp32 precision at full PE rate (products are
        exact; PSUM accumulates in fp32).
      - round(f) in ONE two-scalar DVE op: kt = (f + M) - M with
        M = 1.5*2^23; the intermediate fp32 rounding of (f + M) is
        exactly round-to-nearest (verified exact on HW), so no second
        matmul is needed.  rs = f - kt in [-0.5, 0.5] via DVE stt.
      - sin(2*pi*f) = Sin(2*pi*rs); cos(2*pi*f) = Sin(pi/2 - 2*pi*|rs|)
        with |rs| on ACT Abs (valid in every table set; the HW Sin table
        is only accurate to ~1.26*pi so the fold is mandatory).
  * scores: fp32r matmuls, heads pre-transposed on host to [D, O];
    invalid-mask offsets are added via a K=1 ones-matmul accumulate;
    PSUM eviction to SBUF split DVE/ACT to balance engine load.
  * softmax: Exp with accum_out gives the row sums for free; reciprocal
    on DVE batched over both oc halves; the 1/sum scaling is folded into
    the mix-output eviction.
  * mix: per-b transpose of the exp'd score block via PE transpose
    (c-chunks {128,128,17}), bf16 matmuls against x (x is cast to bf16
    on host - halves the input DMA), mix PSUM tiles span 2 banks so the
    eviction runs as one 1024-col op (fused with softmax normalization
    and bf16 output cast, spread DVE/ACT), out-DMAs in bf16 on the sync
    queue; the host upcasts to f32.
  * ACT table switches (Sin <-> Exp) are limited to 2 per half-problem
    by explicit ordering edges between the sin and exp instruction
    groups, letting batches 0-3 flow through softmax+mix while the
    second half's embedding work is still running.
"""

import math
import time

import ml_dtypes
import numpy as np

import concourse.bacc as bacc
import concourse.tile as tile
from concourse import mybir
from concourse.tile import add_dep_helper

F32 = mybir.dt.float32
F32R = mybir.dt.float32r
BF16 = mybir.dt.bfloat16

B, C, T, O, D = 64, 273, 2048, 256, 2048
NF = 32
NIJ = NF * NF
NCORES = 8
BLOC = B // NCORES
BC = BLOC * C        # 2184
BCPAD = 2304         # 128*18 padded wrap layout for position prep
MARGIN = 0.2
WIDTH = 1.0 + 2.0 * MARGIN

SEG = 274            # padded per-batch segment width (fp32r needs even N)
BCL = BLOC * SEG     # 2192 padded columns
QWL = 2 * SEG        # 548 (quarter = 2 batches, padded)
PI = math.pi
MAGIC = 12582912.0   # 1.5 * 2^23

_CACHE = {}
LAST_RUN_NS = None

# engine-assignment knobs (tuned against TimelineSim)
SC_ON_ACT = False      # score PSUM->SBUF eviction engine
WT_ON_ACT = False      # transpose PSUM->SBUF (weights) eviction engine
ABS_ON_ACT = True      # |rs| via ACT Abs vs DVE stt max(-rs, rs)
MIX_DVE_MOD = 2        # mix eviction i goes to DVE iff (i % MIX_DVE_MOD) != 0
OUT_DMA_SPLIT = True   # alternate out-DMAs scalar/sync (False: all sync)
PAIR_MIX = False       # mix PSUM tiles span 2 banks (1024-col evictions)
PSUM_F, PSUM_SC, PSUM_TP, PSUM_MIX = 3, 1, 1, 3  # pool bufs
# ics whose kt runs on ACT (Copy bias=M); their rs comes out negated, fixed
# by host-negating the matching sin head rows
KT_ACT_ICS = frozenset()
RS_BUFS, TRIG_BUFS, K_BUFS, WT_BUFS, OEV_BUFS, X_BUFS = 5, 7, 6, 6, 2, 3


def _consts():
    p = np.arange(NIJ)
    i = (p // NF).astype(np.float32)
    j = (p % NF).astype(np.float32)
    f6 = np.stack([i, i, i, j, j, j]).astype(ml_dtypes.bfloat16)
    ident = np.eye(128, dtype=np.float32)
    ones1 = np.ones((1, 128), dtype=ml_dtypes.bfloat16)
    return f6, ident, ones1


def build(nc=None, loop_n=1):
    nc = nc or bacc.Bacc("TRN2", target_bir_lowering=False, debug=False,
                         enable_partition_id=False)

    x_in = nc.dram_tensor("x", [BLOC, C, T], BF16, kind="ExternalInput")
    u_in = nc.dram_tensor("u", [6, BCPAD], BF16, kind="ExternalInput")
    offs_in = nc.dram_tensor("offs", [1, BCL], BF16, kind="ExternalInput")
    headsT_in = nc.dram_tensor("headsT", [D, O], F32R, kind="ExternalInput")
    out_dram = nc.dram_tensor("out", [BLOC, O, T], BF16, kind="ExternalOutput")

    f6_np, ident_np, ones_np = _consts()
    f6_dram = nc.inline_tensor(f6_np, "f6c")
    ident_dram = nc.inline_tensor(ident_np, "identc")
    ones_dram = nc.inline_tensor(ones_np, "onesc")

    with tile.TileContext(nc) as tc:
        if loop_n > 1:
            with tc.For_i(0, loop_n):
                _build_tile(tc, x_in, u_in, offs_in, headsT_in, out_dram,
                            f6_dram, ident_dram, ones_dram)
        else:
            _build_tile(tc, x_in, u_in, offs_in, headsT_in, out_dram,
                        f6_dram, ident_dram, ones_dram)
    nc.compile()
    return nc


def _build_tile(tc, x_in, u_in, offs_in, headsT_in, out_dram,
                f6_dram, ident_dram, ones_dram):
    nc = tc.nc
    Sin = mybir.ActivationFunctionType.Sin
    Exp = mybir.ActivationFunctionType.Exp
    Copy = mybir.ActivationFunctionType.Copy
    Abs = mybir.ActivationFunctionType.Abs
    ALU = mybir.AluOpType

    import contextlib
    ctx = contextlib.ExitStack()

    singles = ctx.enter_context(tc.tile_pool(name="singles", bufs=1))
    # U[6, BCPAD]: 3 bf16 limbs of u=(posx+m)/w, 3 of v; host-encoded so
    # the embedding matmuls can start as soon as this one DMA lands.
    # u/f6 ride the scalar-engine HWDGE queue so the first embedding
    # matmuls don't wait behind the 2MB heads DMA on the sync queue.
    u_sb = singles.tile([6, BCPAD], BF16, name="u_sb")
    nc.scalar.dma_start(out=u_sb, in_=u_in.ap())
    f6_sb = singles.tile([6, NIJ], BF16, name="f6_sb")
    nc.scalar.dma_start(out=f6_sb, in_=f6_dram.ap())
    ident_sb = singles.tile([128, 128], F32R, name="ident_sb")
    nc.sync.dma_start(out=ident_sb, in_=ident_dram.ap().bitcast(F32R))
    ones_sb = singles.tile([1, 128], BF16, name="ones_sb")
    nc.sync.dma_start(out=ones_sb, in_=ones_dram.ap())
    offs_sb = singles.tile([1, BCL], BF16, name="offs_sb")
    nc.sync.dma_start(out=offs_sb, in_=offs_in.ap())
    hpi_sb = singles.tile([128, 1], F32, name="hpi_sb")
    nc.vector.memset(hpi_sb, PI / 2)


    # heads, pre-transposed on host: hT[dl, ic*O + o] = headsT[ic*128+dl, o]
    hT = singles.tile([128, 16 * O], F32R, name="hT")
    nc.sync.dma_start(
        out=hT.rearrange("dl (ic o) -> dl ic o", o=O),
        in_=headsT_in.ap().rearrange("(ic dl) o -> dl ic o", dl=128))

    # --- pools ---
    # PSUM budget (8 banks): f + scores + transpose 1 + 2*mix pairs
    f_ps = ctx.enter_context(tc.tile_pool(name="f_ps", bufs=PSUM_F, space="PSUM"))
    sc_ps = ctx.enter_context(tc.tile_pool(name="sc_ps", bufs=PSUM_SC, space="PSUM"))
    tp_ps = ctx.enter_context(tc.tile_pool(name="tp_ps", bufs=PSUM_TP, space="PSUM"))
    mix_ps = ctx.enter_context(tc.tile_pool(name="mix_ps", bufs=PSUM_MIX, space="PSUM"))

    rs_pool = ctx.enter_context(tc.tile_pool(name="rs_pool", bufs=RS_BUFS))
    trig_pool = ctx.enter_context(tc.tile_pool(name="trig_pool", bufs=TRIG_BUFS))
    k_pool = ctx.enter_context(tc.tile_pool(name="k_pool", bufs=K_BUFS))
    sc_sb_pool = ctx.enter_context(tc.tile_pool(name="sc_sb", bufs=1))
    sums_pool = ctx.enter_context(tc.tile_pool(name="sums", bufs=1))
    wt_pool = ctx.enter_context(tc.tile_pool(name="wt", bufs=WT_BUFS))
    x_pool = ctx.enter_context(tc.tile_pool(name="x_pool", bufs=X_BUFS))
    oev_pool = ctx.enter_context(tc.tile_pool(name="oev", bufs=OEV_BUFS))

    SC = [sc_sb_pool.tile([128, BCL], F32R, name=f"SC{oc}") for oc in range(2)]
    sums = sums_pool.tile([128, 2 * BLOC], F32, name="sums")
    rsums = sums_pool.tile([128, 2 * BLOC], F32, name="rsums")

    CW = [(0, 128), (128, 128), (256, C - 256)]

    sin_insts_group = [[], []]
    exp_insts_group = [[], []]
    trig_list = {}
    mix_evict_ctr = [0]

    for g in range(2):
        # ---------- embedding + scores (2 quarters per group) ----------
        for qq in range(2):
            q = g * 2 + qq
            for pc in range(4):      # paired ij-chunks
                # rs tile: [rs block | abs block], each 2*QWL; sin/cos run
                # in place on the rs/abs blocks
                rs_t = rs_pool.tile([128, 4 * QWL], F32, tag="rs")
                for half in range(2):
                    ic = 2 * pc + half
                    kt_on_act = ic in KT_ACT_ICS
                    for bi in range(2):
                        ucol = q * QWL + bi * SEG
                        fp = f_ps.tile([128, SEG], F32, tag="f", name="fp")
                        nc.tensor.matmul(
                            fp,
                            f6_sb[:, ic * 128:(ic + 1) * 128],
                            u_sb[:, ucol:ucol + SEG],
                            start=True, stop=True)
                        col = half * QWL + bi * SEG
                        kt = k_pool.tile([128, SEG], F32, tag="kt", name="kt")
                        if kt_on_act:
                            # kt_pre = RN(f + M) = M + round(f) on ACT (the
                            # fp32 output write rounds); rs' = -rs, absorbed
                            # by host-negating this ic's sin head rows
                            nc.scalar.activation(kt, fp, Copy, bias=MAGIC)
                            nc.vector.scalar_tensor_tensor(
                                rs_t[:, col:col + SEG], kt, MAGIC, fp,
                                ALU.subtract, ALU.subtract)
                        else:
                            # kt = round(f): fp32 rounding of (f + M) - M is
                            # exact round-to-nearest for |f| << 2^22
                            nc.vector.tensor_scalar(
                                kt, fp, MAGIC, MAGIC, ALU.add, ALU.subtract)
                            # rs = f - round(f)  in [-0.5, 0.5], exact
                            nc.vector.scalar_tensor_tensor(
                                rs_t[:, col:col + SEG], fp, 0.0, kt,
                                ALU.add, ALU.subtract)
                # |rs| for the cos fold (Sin table is only valid to ~1.26pi)
                if ABS_ON_ACT:
                    nc.scalar.activation(rs_t[:, 2 * QWL:], rs_t[:, :2 * QWL],
                                         Abs)
                else:
                    nc.vector.scalar_tensor_tensor(
                        rs_t[:, 2 * QWL:], rs_t[:, :2 * QWL], -1.0,
                        rs_t[:, :2 * QWL], ALU.mult, ALU.max)
                # separate F32R tile: the fp32r score matmuls require their
                # operand bytes to be produced f32r-rounded (BIR verifier
                # checks every overlapping writer, so in-place is out)
                trig_t = trig_pool.tile([128, 4 * QWL], F32R, tag="trig",
                                        name=f"trig_q{q}p{pc}")
                si = nc.scalar.activation(trig_t[:, :2 * QWL],
                                          rs_t[:, :2 * QWL], Sin,
                                          bias=0.0, scale=2 * PI)
                ci = nc.scalar.activation(trig_t[:, 2 * QWL:],
                                          rs_t[:, 2 * QWL:], Sin,
                                          bias=hpi_sb, scale=-2 * PI)
                sin_insts_group[g] += [si, ci]
                if g == 1:
                    for e in exp_insts_group[0]:
                        add_dep_helper(si.ins, e.ins, sync=False,
                                       reason="ACT order: g1 sins after g0 exps")
                        add_dep_helper(ci.ins, e.ins, sync=False,
                                       reason="ACT order: g1 sins after g0 exps")
                trig_list[(q, pc)] = trig_t

            for oc in range(2):
                for bi in range(2):
                    b = q * 2 + bi
                    sp = sc_ps.tile([128, SEG], F32, tag="sc", name="sp")
                    first = True
                    for pc in range(4):
                        trig_t = trig_list[(q, pc)]
                        for half in range(2):
                            ic = 2 * pc + half
                            col = half * QWL + bi * SEG
                            # cos block (cols 2*QWL..), then sin block
                            nc.tensor.matmul(
                                sp,
                                hT[:, ic * O + oc * 128: ic * O + oc * 128 + 128],
                                trig_t[:, 2 * QWL + col:2 * QWL + col + SEG],
                                start=first, stop=False, skip_group_check=True)
                            first = False
                            nc.tensor.matmul(
                                sp,
                                hT[:, (8 + ic) * O + oc * 128: (8 + ic) * O + oc * 128 + 128],
                                trig_t[:, col:col + SEG],
                                start=False, stop=False, skip_group_check=True)
                    nc.tensor.matmul(
                        sp, ones_sb,
                        offs_sb[:, b * SEG:b * SEG + SEG],
                        start=False, stop=True, skip_group_check=True)
                    if SC_ON_ACT:
                        nc.scalar.activation(
                            SC[oc][:, b * SEG:b * SEG + SEG],
                            sp.bitcast(F32R), Copy)
                    else:
                        nc.vector.tensor_copy(
                            SC[oc][:, b * SEG:b * SEG + SEG],
                            sp.bitcast(F32R))

        # ---------- softmax + mix (4 batches per group) ----------
        for bi in range(4):
            b = g * 4 + bi
            seg = slice(b * SEG, b * SEG + C)
            for oc in range(2):
                ei = nc.scalar.activation(
                    SC[oc][:, seg], SC[oc][:, seg], Exp,
                    accum_out=sums[:, 2 * b + oc: 2 * b + oc + 1])
                for s in sin_insts_group[g]:
                    add_dep_helper(ei.ins, s.ins, sync=False,
                                   reason="ACT order: exps after group sins")
                exp_insts_group[g].append(ei)
            # both oc sums are adjacent: one reciprocal per batch
            nc.vector.reciprocal(
                rsums[:, 2 * b: 2 * b + 2],
                sums[:, 2 * b: 2 * b + 2])

            wts = []
            for kc, (c0, cw) in enumerate(CW):
                wt = wt_pool.tile([128, O], BF16, tag=f"wt{kc}")
                wts.append(wt)
                tp = tp_ps.tile([128, O], F32R, tag="tp", name="tp")
                for oc in range(2):
                    nc.tensor.transpose(
                        tp[:cw, oc * 128:(oc + 1) * 128],
                        SC[oc][:, b * SEG + c0: b * SEG + c0 + cw],
                        ident_sb)
                if WT_ON_ACT:
                    nc.scalar.activation(wt[:cw, :], tp[:cw, :], Copy)
                else:
                    nc.vector.tensor_copy(wt[:cw, :], tp[:cw, :])

            xts = []
            for kc, (c0, cw) in enumerate(CW):
                xt = x_pool.tile([128, T], BF16, tag=f"x{kc}")
                xts.append(xt)
                nc.sync.dma_start(out=xt[:cw, :], in_=x_in.ap()[b, c0:c0 + cw, :])

            def _mix_evict(oslice, op, rsum_col):
                i = mix_evict_ctr[0]
                mix_evict_ctr[0] += 1
                # fused softmax-normalization + bf16 cast eviction,
                # spread DVE/ACT (Pool cannot read PSUM)
                if i % MIX_DVE_MOD == 0:
                    nc.scalar.activation(oslice, op, Copy,
                                         bias=0.0, scale=rsum_col)
                else:
                    nc.vector.tensor_scalar(oslice, op, rsum_col,
                                            None, ALU.mult)

            for oc in range(2):
                oe = oev_pool.tile([128, T], BF16, tag="oe")
                rsum_col = rsums[:, 2 * b + oc: 2 * b + oc + 1]
                if PAIR_MIX:
                    for tp2 in range(2):
                        # two-bank PSUM tile: both tt halves land here so
                        # the eviction runs as a single 1024-col op
                        op = mix_ps.tile([128, 1024], F32, tag="mo", name="mop")
                        for tth in range(2):
                            tt = tp2 * 2 + tth
                            for kc, (c0, cw) in enumerate(CW):
                                nc.tensor.matmul(
                                    op[:, tth * 512:(tth + 1) * 512],
                                    wts[kc][:cw, oc * 128:(oc + 1) * 128],
                                    xts[kc][:cw, tt * 512:(tt + 1) * 512],
                                    start=(kc == 0), stop=(kc == 2),
                                    skip_group_check=True)
                        _mix_evict(oe[:, tp2 * 1024:(tp2 + 1) * 1024],
                                   op, rsum_col)
                else:
                    for tt in range(4):
                        op = mix_ps.tile([128, 512], F32, tag="mo", name="mop")
                        for kc, (c0, cw) in enumerate(CW):
                            nc.tensor.matmul(
                                op,
                                wts[kc][:cw, oc * 128:(oc + 1) * 128],
                                xts[kc][:cw, tt * 512:(tt + 1) * 512],
                                start=(kc == 0), stop=(kc == 2),
                                skip_group_check=True)
                        _mix_evict(oe[:, tt * 512:(tt + 1) * 512],
                                   op, rsum_col)
                # two half-DMAs so the first overlaps the tail evictions
                out_eng = (nc.scalar if (OUT_DMA_SPLIT and b % 2 == 0)
                           else nc.sync)
                for hh in range(2):
                    out_eng.dma_start(
                        out=out_dram.ap()[b, oc * 128:(oc + 1) * 128,
                                          hh * 1024:(hh + 1) * 1024],
                        in_=oe[:, hh * 1024:(hh + 1) * 1024])

    ctx.close()


# --------------------------------------------------------------------------
# host side
# --------------------------------------------------------------------------

def _host_inputs(x, positions, invalid_mask, heads):
    headsT = np.ascontiguousarray(np.asarray(heads, dtype=np.float32).T)
    if KT_ACT_ICS:
        headsT = headsT.copy()
        for ic in KT_ACT_ICS:
            # these ics' rs is negated on device (kt-on-ACT path); absorb
            # the sign into the sin head rows (cos is even, unaffected)
            headsT[NIJ + ic * 128: NIJ + (ic + 1) * 128, :] *= -1.0
    x_bf = np.asarray(x, dtype=np.float32).astype(ml_dtypes.bfloat16)
    in_maps = []
    for core in range(NCORES):
        bsl = slice(core * BLOC, (core + 1) * BLOC)
        xs = np.ascontiguousarray(x_bf[bsl])

        pos = positions[bsl].reshape(BLOC, C, 2).astype(np.float64)
        uvw = np.zeros((2, BCPAD), dtype=np.float64)
        for ax in range(2):
            seg = uvw[ax, :BCL].reshape(BLOC, SEG)
            seg[:, :C] = (pos[:, :, ax] + MARGIN) / WIDTH
        u6 = np.zeros((6, BCPAD), dtype=ml_dtypes.bfloat16)
        for ax in range(2):
            resid = uvw[ax].copy()
            for li in range(3):
                limb = resid.astype(ml_dtypes.bfloat16)
                u6[3 * ax + li] = limb
                resid = resid - limb.astype(np.float64)

        offs = np.zeros((1, BCL), dtype=np.float32)
        offs[0, :].reshape(BLOC, SEG)[:, :C] = np.where(
            invalid_mask[bsl], -1e30, 0.0)
        offs = offs.astype(ml_dtypes.bfloat16)

        in_maps.append({"x": xs, "u": u6, "offs": offs, "headsT": headsT})
    return in_maps


def kernel(**inputs):
    global LAST_RUN_NS
    from concourse.bass_utils import run_bass_kernel_spmd

    x = np.asarray(inputs["x"])
    positions = np.asarray(inputs["positions"])
    invalid_mask = np.asarray(inputs["invalid_mask"])
    heads = np.asarray(inputs["heads"])

    if "nc" not in _CACHE:
        _CACHE["nc"] = build()
    nc = _CACHE["nc"]

    in_maps = _host_inputs(x, positions, invalid_mask, heads)
    t0 = time.perf_counter()
    res = run_bass_kernel_spmd(nc, in_maps, core_ids=list(range(NCORES)))
    LAST_RUN_NS = (time.perf_counter() - t0) * 1e9
    out = np.concatenate([r["out"] for r in res.results], axis=0)
    return out.astype(np.float32)


# revision 32
# speedup vs baseline: 502.9362x; 1.5577x over previous
"""Trainium2 Bass/Tile kernel for nn_ChannelMerger.

Reference computation (per batch b):
    emb[c, d]   = fourier_embedding(positions[c])          # d = 2048
    scores[o,c] = sum_d emb[c,d] * heads[o,d] + offset[c]
    w[o,c]      = softmax_c(scores)
    out[o,t]    = sum_c x[c,t] * w[o,c]

Shapes: B=64, C=273, T=2048, O=256, D=2048 (n_freqs=32).
Sharding: data-parallel over B across 8 cores (8 batches per core).
The bc axis is laid out in 274-wide per-batch segments (fp32r matmuls
need an even moving-dim; the pad column carries zeros end to end).

Device algorithm (per core):
  * turns-domain fourier embedding, f[ij, bc] = i*u[bc] + j*v[bc] with
    u = (posx+margin)/width, v likewise:
      - u, v are encoded host-side into 3 bf16 limbs each (a lossless
        precision split of the 4K rescaled position scalars), so a K=6
        bf16 matmul against the exact integer rows [i,i,i,j,j,j]
        reproduces f at ~fp32 precision at full PE rate (products are
        exact; PSUM accumulates in fp32).
      - round(f) in ONE two-scalar DVE op: kt = (f + M) - M with
        M = 1.5*2^23; the intermediate fp32 rounding of (f + M) is
        exactly round-to-nearest (verified exact on HW), so no second
        matmul is needed.  rs = f - kt in [-0.5, 0.5] via DVE stt.
      - sin(2*pi*f) = Sin(2*pi*rs); cos(2*pi*f) = Sin(pi/2 - 2*pi*|rs|)
        with |rs| on ACT Abs (valid in every table set; the HW Sin table
        is only accurate to ~1.26*pi so the fold is mandatory).
  * scores: fp32r matmuls, heads pre-transposed on host to [D, O];
    invalid-mask offsets are added via a K=1 ones-matmul accumulate;
    PSUM eviction to SBUF split DVE/ACT to balance engine load.
  * softmax: Exp with accum_out gives the row sums for free; reciprocal
    on DVE batched over both oc halves; the 1/sum scaling is folded into
    the mix-output eviction.
  * mix: per-b transpose of the exp'd score block via PE transpose
    (c-chunks {128,128,17}), bf16 matmuls against x (x is cast to bf16
    on host - halves the input DMA), mix PSUM tiles span 2 banks so the
    eviction runs as one 1024-col op (fused with softmax normalization
    and bf16 output cast, spread DVE/ACT), out-DMAs in bf16 on the sync
    queue; the host upcasts to f32.
  * ACT table switches (Sin <-> Exp) are limited to 2 per half-problem
    by explicit ordering edges between the sin and exp instruction
    groups, letting batches 0-3 flow through softmax+mix while the
    second half's embedding work is still running.
"""

import math
import time

import ml_dtypes
import numpy as np

import concourse.bacc as bacc
import concourse.tile as tile
from concourse import mybir
from concourse.tile import add_dep_helper

F32 = mybir.dt.float32
F32R = mybir.dt.float32r
BF16 = mybir.dt.bfloat16

B, C, T, O, D = 64, 273, 2048, 256, 2048
NF = 32
NIJ = NF * NF
NCORES = 8
BLOC = B // NCORES
BC = BLOC * C        # 2184
BCPAD = 2304         # 128*18 padded wrap layout for position prep
MARGIN = 0.2
WIDTH = 1.0 + 2.0 * MARGIN

SEG = 274            # padded per-batch segment width (fp32r needs even N)
BCL = BLOC * SEG     # 2192 padded columns
QWL = 2 * SEG        # 548 (quarter = 2 batches, padded)
PI = math.pi
MAGIC = 12582912.0   # 1.5 * 2^23

_CACHE = {}
LAST_RUN_NS = None

# engine-assignment knobs (tuned against TimelineSim)
SC_ON_ACT = False      # score PSUM->SBUF eviction engine
WT_ON_ACT = False      # transpose PSUM->SBUF (weights) eviction engine
ABS_ON_ACT = True      # |rs| via ACT Abs vs DVE stt max(-rs, rs)
MIX_DVE_MOD = 2        # mix eviction i goes to DVE iff (i % MIX_DVE_MOD) != 0
OUT_DMA_SPLIT = False  # alternate out-DMAs scalar/sync (False: all sync)
OUT_ONE_DMA = True     # one [128,2048] out-DMA per (b,oc) instead of two halves
PAIR_MIX = False       # mix PSUM tiles span 2 banks (1024-col evictions)
PSUM_F, PSUM_SC, PSUM_TP, PSUM_MIX = 3, 1, 1, 3  # pool bufs
# ics whose kt runs on ACT (Copy bias=M); their rs comes out negated, fixed
# by host-negating the matching sin head rows
KT_ACT_ICS = frozenset()
RS_BUFS, TRIG_BUFS, K_BUFS, WT_BUFS, OEV_BUFS, X_BUFS = 5, 7, 6, 6, 2, 3
STAGGERED_LOOP = False   # For_i staggered semaphore reset (timing loop only)


def _consts():
    p = np.arange(NIJ)
    i = (p // NF).astype(np.float32)
    j = (p % NF).astype(np.float32)
    f6 = np.stack([i, i, i, j, j, j]).astype(ml_dtypes.bfloat16)
    ident = np.eye(128, dtype=np.float32)
    ones1 = np.ones((1, 128), dtype=ml_dtypes.bfloat16)
    return f6, ident, ones1


def build(nc=None, loop_n=1):
    nc = nc or bacc.Bacc("TRN2", target_bir_lowering=False, debug=False,
                         enable_partition_id=False)

    x_in = nc.dram_tensor("x", [BLOC, C, T], BF16, kind="ExternalInput")
    u_in = nc.dram_tensor("u", [6, BCPAD], BF16, kind="ExternalInput")
    offs_in = nc.dram_tensor("offs", [1, BCL], BF16, kind="ExternalInput")
    headsT_in = nc.dram_tensor("headsT", [D, O], F32R, kind="ExternalInput")
    out_dram = nc.dram_tensor("out", [BLOC, O, T], BF16, kind="ExternalOutput")

    f6_np, ident_np, ones_np = _consts()
    f6_dram = nc.inline_tensor(f6_np, "f6c")
    ident_dram = nc.inline_tensor(ident_np, "identc")
    ones_dram = nc.inline_tensor(ones_np, "onesc")

    with tile.TileContext(nc) as tc:
        if loop_n > 1:
            with tc.For_i(0, loop_n, staggered_reset=STAGGERED_LOOP):
                _build_tile(tc, x_in, u_in, offs_in, headsT_in, out_dram,
                            f6_dram, ident_dram, ones_dram)
        else:
            _build_tile(tc, x_in, u_in, offs_in, headsT_in, out_dram,
                        f6_dram, ident_dram, ones_dram)
    nc.compile()
    return nc


def _build_tile(tc, x_in, u_in, offs_in, headsT_in, out_dram,
                f6_dram, ident_dram, ones_dram):
    nc = tc.nc
    Sin = mybir.ActivationFunctionType.Sin
    Exp = mybir.ActivationFunctionType.Exp
    Copy = mybir.ActivationFunctionType.Copy
    Abs = mybir.ActivationFunctionType.Abs
    ALU = mybir.AluOpType

    import contextlib
    ctx = contextlib.ExitStack()

    singles = ctx.enter_context(tc.tile_pool(name="singles", bufs=1))
    # U[6, BCPAD]: 3 bf16 limbs of u=(posx+m)/w, 3 of v; host-encoded so
    # the embedding matmuls can start as soon as this one DMA lands.
    # u/f6 ride the scalar-engine HWDGE queue so the first embedding
    # matmuls don't wait behind the 2MB heads DMA on the sync queue.
    u_sb = singles.tile([6, BCPAD], BF16, name="u_sb")
    nc.scalar.dma_start(out=u_sb, in_=u_in.ap())
    f6_sb = singles.tile([6, NIJ], BF16, name="f6_sb")
    nc.scalar.dma_start(out=f6_sb, in_=f6_dram.ap())
    ident_sb = singles.tile([128, 128], F32R, name="ident_sb")
    nc.sync.dma_start(out=ident_sb, in_=ident_dram.ap().bitcast(F32R))
    ones_sb = singles.tile([1, 128], BF16, name="ones_sb")
    nc.sync.dma_start(out=ones_sb, in_=ones_dram.ap())
    offs_sb = singles.tile([1, BCL], BF16, name="offs_sb")
    nc.sync.dma_start(out=offs_sb, in_=offs_in.ap())
    hpi_sb = singles.tile([128, 1], F32, name="hpi_sb")
    nc.vector.memset(hpi_sb, PI / 2)


    # heads, pre-transposed on host: hT[dl, ic*O + o] = headsT[ic*128+dl, o]
    hT = singles.tile([128, 16 * O], F32R, name="hT")
    nc.sync.dma_start(
        out=hT.rearrange("dl (ic o) -> dl ic o", o=O),
        in_=headsT_in.ap().rearrange("(ic dl) o -> dl ic o", dl=128))

    # --- pools ---
    # PSUM budget (8 banks): f + scores + transpose 1 + 2*mix pairs
    f_ps = ctx.enter_context(tc.tile_pool(name="f_ps", bufs=PSUM_F, space="PSUM"))
    sc_ps = ctx.enter_context(tc.tile_pool(name="sc_ps", bufs=PSUM_SC, space="PSUM"))
    tp_ps = ctx.enter_context(tc.tile_pool(name="tp_ps", bufs=PSUM_TP, space="PSUM"))
    mix_ps = ctx.enter_context(tc.tile_pool(name="mix_ps", bufs=PSUM_MIX, space="PSUM"))

    rs_pool = ctx.enter_context(tc.tile_pool(name="rs_pool", bufs=RS_BUFS))
    trig_pool = ctx.enter_context(tc.tile_pool(name="trig_pool", bufs=TRIG_BUFS))
    k_pool = ctx.enter_context(tc.tile_pool(name="k_pool", bufs=K_BUFS))
    sc_sb_pool = ctx.enter_context(tc.tile_pool(name="sc_sb", bufs=1))
    sums_pool = ctx.enter_context(tc.tile_pool(name="sums", bufs=1))
    wt_pool = ctx.enter_context(tc.tile_pool(name="wt", bufs=WT_BUFS))
    x_pool = ctx.enter_context(tc.tile_pool(name="x_pool", bufs=X_BUFS))
    oev_pool = ctx.enter_context(tc.tile_pool(name="oev", bufs=OEV_BUFS))

    SC = [sc_sb_pool.tile([128, BCL], F32R, name=f"SC{oc}") for oc in range(2)]
    sums = sums_pool.tile([128, 2 * BLOC], F32, name="sums")
    rsums = sums_pool.tile([128, 2 * BLOC], F32, name="rsums")

    CW = [(0, 128), (128, 128), (256, C - 256)]

    sin_insts_group = [[], []]
    exp_insts_group = [[], []]
    trig_list = {}
    mix_evict_ctr = [0]

    for g in range(2):
        # ---------- embedding + scores (2 quarters per group) ----------
        for qq in range(2):
            q = g * 2 + qq
            for pc in range(4):      # paired ij-chunks
                # rs tile: [rs block | abs block], each 2*QWL; sin/cos run
                # in place on the rs/abs blocks
                rs_t = rs_pool.tile([128, 4 * QWL], F32, tag="rs")
                for half in range(2):
                    ic = 2 * pc + half
                    kt_on_act = ic in KT_ACT_ICS
                    for bi in range(2):
                        ucol = q * QWL + bi * SEG
                        fp = f_ps.tile([128, SEG], F32, tag="f", name="fp")
                        nc.tensor.matmul(
                            fp,
                            f6_sb[:, ic * 128:(ic + 1) * 128],
                            u_sb[:, ucol:ucol + SEG],
                            start=True, stop=True)
                        col = half * QWL + bi * SEG
                        kt = k_pool.tile([128, SEG], F32, tag="kt", name="kt")
                        if kt_on_act:
                            # kt_pre = RN(f + M) = M + round(f) on ACT (the
                            # fp32 output write rounds); rs' = -rs, absorbed
                            # by host-negating this ic's sin head rows
                            nc.scalar.activation(kt, fp, Copy, bias=MAGIC)
                            nc.vector.scalar_tensor_tensor(
                                rs_t[:, col:col + SEG], kt, MAGIC, fp,
                                ALU.subtract, ALU.subtract)
                        else:
                            # kt = round(f): fp32 rounding of (f + M) - M is
                            # exact round-to-nearest for |f| << 2^22
                            nc.vector.tensor_scalar(
                                kt, fp, MAGIC, MAGIC, ALU.add, ALU.subtract)
                            # rs = f - round(f)  in [-0.5, 0.5], exact
                            nc.vector.scalar_tensor_tensor(
                                rs_t[:, col:col + SEG], fp, 0.0, kt,
                                ALU.add, ALU.subtract)
                # |rs| for the cos fold (Sin table is only valid to ~1.26pi)
                if ABS_ON_ACT:
                    nc.scalar.activation(rs_t[:, 2 * QWL:], rs_t[:, :2 * QWL],
                                         Abs)
                else:
                    nc.vector.scalar_tensor_tensor(
                        rs_t[:, 2 * QWL:], rs_t[:, :2 * QWL], -1.0,
                        rs_t[:, :2 * QWL], ALU.mult, ALU.max)
                # separate F32R tile: the fp32r score matmuls require their
                # operand bytes to be produced f32r-rounded (BIR verifier
                # checks every overlapping writer, so in-place is out)
                trig_t = trig_pool.tile([128, 4 * QWL], F32R, tag="trig",
                                        name=f"trig_q{q}p{pc}")
                si = nc.scalar.activation(trig_t[:, :2 * QWL],
                                          rs_t[:, :2 * QWL], Sin,
                                          bias=0.0, scale=2 * PI)
                ci = nc.scalar.activation(trig_t[:, 2 * QWL:],
                                          rs_t[:, 2 * QWL:], Sin,
                                          bias=hpi_sb, scale=-2 * PI)
                sin_insts_group[g] += [si, ci]
                if g == 1:
                    for e in exp_insts_group[0]:
                        add_dep_helper(si.ins, e.ins, sync=False,
                                       reason="ACT order: g1 sins after g0 exps")
                        add_dep_helper(ci.ins, e.ins, sync=False,
                                       reason="ACT order: g1 sins after g0 exps")
                trig_list[(q, pc)] = trig_t

            for oc in range(2):
                for bi in range(2):
                    b = q * 2 + bi
                    sp = sc_ps.tile([128, SEG], F32, tag="sc", name="sp")
                    first = True
                    for pc in range(4):
                        trig_t = trig_list[(q, pc)]
                        for half in range(2):
                            ic = 2 * pc + half
                            col = half * QWL + bi * SEG
                            # cos block (cols 2*QWL..), then sin block
                            nc.tensor.matmul(
                                sp,
                                hT[:, ic * O + oc * 128: ic * O + oc * 128 + 128],
                                trig_t[:, 2 * QWL + col:2 * QWL + col + SEG],
                                start=first, stop=False, skip_group_check=True)
                            first = False
                            nc.tensor.matmul(
                                sp,
                                hT[:, (8 + ic) * O + oc * 128: (8 + ic) * O + oc * 128 + 128],
                                trig_t[:, col:col + SEG],
                                start=False, stop=False, skip_group_check=True)
                    nc.tensor.matmul(
                        sp, ones_sb,
                        offs_sb[:, b * SEG:b * SEG + SEG],
                        start=False, stop=True, skip_group_check=True)
                    if SC_ON_ACT:
                        nc.scalar.activation(
                            SC[oc][:, b * SEG:b * SEG + SEG],
                            sp.bitcast(F32R), Copy)
                    else:
                        nc.vector.tensor_copy(
                            SC[oc][:, b * SEG:b * SEG + SEG],
                            sp.bitcast(F32R))

        # ---------- softmax + mix (4 batches per group) ----------
        for bi in range(4):
            b = g * 4 + bi
            seg = slice(b * SEG, b * SEG + C)
            for oc in range(2):
                ei = nc.scalar.activation(
                    SC[oc][:, seg], SC[oc][:, seg], Exp,
                    accum_out=sums[:, 2 * b + oc: 2 * b + oc + 1])
                for s in sin_insts_group[g]:
                    add_dep_helper(ei.ins, s.ins, sync=False,
                                   reason="ACT order: exps after group sins")
                exp_insts_group[g].append(ei)
            # both oc sums are adjacent: one reciprocal per batch
            nc.vector.reciprocal(
                rsums[:, 2 * b: 2 * b + 2],
                sums[:, 2 * b: 2 * b + 2])

            wts = []
            for kc, (c0, cw) in enumerate(CW):
                wt = wt_pool.tile([128, O], BF16, tag=f"wt{kc}")
                wts.append(wt)
                tp = tp_ps.tile([128, O], F32R, tag="tp", name="tp")
                for oc in range(2):
                    nc.tensor.transpose(
                        tp[:cw, oc * 128:(oc + 1) * 128],
                        SC[oc][:, b * SEG + c0: b * SEG + c0 + cw],
                        ident_sb)
                if WT_ON_ACT:
                    nc.scalar.activation(wt[:cw, :], tp[:cw, :], Copy)
                else:
                    nc.vector.tensor_copy(wt[:cw, :], tp[:cw, :])

            xts = []
            for kc, (c0, cw) in enumerate(CW):
                xt = x_pool.tile([128, T], BF16, tag=f"x{kc}")
                xts.append(xt)
                nc.sync.dma_start(out=xt[:cw, :], in_=x_in.ap()[b, c0:c0 + cw, :])

            def _mix_evict(oslice, op, rsum_col):
                i = mix_evict_ctr[0]
                mix_evict_ctr[0] += 1
                # fused softmax-normalization + bf16 cast eviction,
                # spread DVE/ACT (Pool cannot read PSUM)
                if i % MIX_DVE_MOD == 0:
                    nc.scalar.activation(oslice, op, Copy,
                                         bias=0.0, scale=rsum_col)
                else:
                    nc.vector.tensor_scalar(oslice, op, rsum_col,
                                            None, ALU.mult)

            for oc in range(2):
                oe = oev_pool.tile([128, T], BF16, tag="oe")
                rsum_col = rsums[:, 2 * b + oc: 2 * b + oc + 1]
                if PAIR_MIX:
                    for tp2 in range(2):
                        # two-bank PSUM tile: both tt halves land here so
                        # the eviction runs as a single 1024-col op
                        op = mix_ps.tile([128, 1024], F32, tag="mo", name="mop")
                        for tth in range(2):
                            tt = tp2 * 2 + tth
                            for kc, (c0, cw) in enumerate(CW):
                                nc.tensor.matmul(
                                    op[:, tth * 512:(tth + 1) * 512],
                                    wts[kc][:cw, oc * 128:(oc + 1) * 128],
                                    xts[kc][:cw, tt * 512:(tt + 1) * 512],
                                    start=(kc == 0), stop=(kc == 2),
                                    skip_group_check=True)
                        _mix_evict(oe[:, tp2 * 1024:(tp2 + 1) * 1024],
                                   op, rsum_col)
                else:
                    for tt in range(4):
                        op = mix_ps.tile([128, 512], F32, tag="mo", name="mop")
                        for kc, (c0, cw) in enumerate(CW):
                            nc.tensor.matmul(
                                op,
                                wts[kc][:cw, oc * 128:(oc + 1) * 128],
                                xts[kc][:cw, tt * 512:(tt + 1) * 512],
                                start=(kc == 0), stop=(kc == 2),
                                skip_group_check=True)
                        _mix_evict(oe[:, tt * 512:(tt + 1) * 512],
                                   op, rsum_col)
                # two half-DMAs so the first overlaps the tail evictions
                out_eng = (nc.scalar if (OUT_DMA_SPLIT and b % 2 == 0)
                           else nc.sync)
                if OUT_ONE_DMA:
                    out_eng.dma_start(
                        out=out_dram.ap()[b, oc * 128:(oc + 1) * 128, :],
                        in_=oe)
                else:
                    for hh in range(2):
                        out_eng.dma_start(
                            out=out_dram.ap()[b, oc * 128:(oc + 1) * 128,
                                              hh * 1024:(hh + 1) * 1024],
                            in_=oe[:, hh * 1024:(hh + 1) * 1024])

    ctx.close()


# --------------------------------------------------------------------------
# host side
# --------------------------------------------------------------------------

def _host_inputs(x, positions, invalid_mask, heads):
    headsT = np.ascontiguousarray(np.asarray(heads, dtype=np.float32).T)
    if KT_ACT_ICS:
        headsT = headsT.copy()
        for ic in KT_ACT_ICS:
            # these ics' rs is negated on device (kt-on-ACT path); absorb
            # the sign into the sin head rows (cos is even, unaffected)
            headsT[NIJ + ic * 128: NIJ + (ic + 1) * 128, :] *= -1.0
    x_bf = np.asarray(x, dtype=np.float32).astype(ml_dtypes.bfloat16)
    in_maps = []
    for core in range(NCORES):
        bsl = slice(core * BLOC, (core + 1) * BLOC)
        xs = np.ascontiguousarray(x_bf[bsl])

        pos = positions[bsl].reshape(BLOC, C, 2).astype(np.float64)
        uvw = np.zeros((2, BCPAD), dtype=np.float64)
        for ax in range(2):
            seg = uvw[ax, :BCL].reshape(BLOC, SEG)
            seg[:, :C] = (pos[:, :, ax] + MARGIN) / WIDTH
        u6 = np.zeros((6, BCPAD), dtype=ml_dtypes.bfloat16)
        for ax in range(2):
            resid = uvw[ax].copy()
            for li in range(3):
                limb = resid.astype(ml_dtypes.bfloat16)
                u6[3 * ax + li] = limb
                resid = resid - limb.astype(np.float64)

        offs = np.zeros((1, BCL), dtype=np.float32)
        offs[0, :].reshape(BLOC, SEG)[:, :C] = np.where(
            invalid_mask[bsl], -1e30, 0.0)
        offs = offs.astype(ml_dtypes.bfloat16)

        in_maps.append({"x": xs, "u": u6, "offs": offs, "headsT": headsT})
    return in_maps


def kernel(**inputs):
    global LAST_RUN_NS
    from concourse.bass_utils import run_bass_kernel_spmd

    x = np.asarray(inputs["x"])
    positions = np.asarray(inputs["positions"])
    invalid_mask = np.asarray(inputs["invalid_mask"])
    heads = np.asarray(inputs["heads"])

    if "nc" not in _CACHE:
        _CACHE["nc"] = build()
    nc = _CACHE["nc"]

    in_maps = _host_inputs(x, positions, invalid_mask, heads)
    t0 = time.perf_counter()
    res = run_bass_kernel_spmd(nc, in_maps, core_ids=list(range(NCORES)))
    LAST_RUN_NS = (time.perf_counter() - t0) * 1e9
    out = np.concatenate([r["out"] for r in res.results], axis=0)
    return out.astype(np.float32)


# revision 41
# speedup vs baseline: 508.6504x; 1.0114x over previous
"""Trainium2 Bass/Tile kernel for nn_ChannelMerger.

Reference computation (per batch b):
    emb[c, d]   = fourier_embedding(positions[c])          # d = 2048
    scores[o,c] = sum_d emb[c,d] * heads[o,d] + offset[c]
    w[o,c]      = softmax_c(scores)
    out[o,t]    = sum_c x[c,t] * w[o,c]

Shapes: B=64, C=273, T=2048, O=256, D=2048 (n_freqs=32).
Sharding: data-parallel over B across 8 cores (8 batches per core).
The bc axis is laid out in 274-wide per-batch segments (fp32r matmuls
need an even moving-dim; the pad column carries zeros end to end).

Device algorithm (per core):
  * turns-domain fourier embedding, f[ij, bc] = i*u[bc] + j*v[bc] with
    u = (posx+margin)/width, v likewise:
      - u, v are encoded host-side into 3 bf16 limbs each (a lossless
        precision split of the 4K rescaled position scalars), so a K=6
        bf16 matmul against the exact integer rows [i,i,i,j,j,j]
        reproduces f at ~fp32 precision at full PE rate (products are
        exact; PSUM accumulates in fp32).
      - round(f) in ONE two-scalar DVE op: kt = (f + M) - M with
        M = 1.5*2^23; the intermediate fp32 rounding of (f + M) is
        exactly round-to-nearest (verified exact on HW), so no second
        matmul is needed.  rs = f - kt in [-0.5, 0.5] via DVE stt.
      - sin(2*pi*f) = Sin(2*pi*rs); cos(2*pi*f) = Sin(pi/2 - 2*pi*|rs|)
        with |rs| on ACT Abs (valid in every table set; the HW Sin table
        is only accurate to ~1.26*pi so the fold is mandatory).
  * scores: fp32r matmuls, heads pre-transposed on host to [D, O];
    invalid-mask offsets are added via a K=1 ones-matmul accumulate;
    PSUM eviction to SBUF on DVE (ACT's depth-0 in-order queue makes it
    a bad host for dependency-waiting ops - HW-measured).
  * softmax: Exp with accum_out gives the row sums for free; one DVE
    reciprocal per batch covers both oc halves; the 1/sum scaling is
    folded into the mix-output eviction.
  * mix: per-b transpose of the exp'd score block via PE transpose
    (c-chunks {128,128,17}), bf16 matmuls against x (x is cast to bf16
    on host - halves the input DMA), PSUM eviction fused with softmax
    normalization and bf16 output cast (alternating DVE/ACT), one
    [128,2048] bf16 out-DMA per (b,oc) on the sync queue (scalar-queue
    DMAs stall the ACT sequencer - HW-measured); the host upcasts the
    output to f32.
  * ACT table switches (Sin <-> Exp) are limited to 2 per half-problem
    by explicit ordering edges between the sin and exp instruction
    groups, letting batches 0-3 flow through softmax+mix while the
    second half's embedding work is still running.
  * build(loop_n=R) wraps the body in a For_i hardware loop (used by
    test.py to measure per-iteration HW time through the axon tunnel,
    whose fixed ~40-80ms RPC cost swamps any single dispatch).

Engine budget per core (cost model): PE ~85us (scores 31 + mix 41 +
embedding 7 + transposes), DVE ~88us (kt/rs round chain + evictions),
ACT ~85us (sin/cos/abs/exp + evictions) - a balanced three-way split;
HW-measured per-iteration time ~160us (vs ~197us for the previous
kernel, same methodology).
"""

import math
import time

import ml_dtypes
import numpy as np

import concourse.bacc as bacc
import concourse.tile as tile
from concourse import mybir
from concourse.tile import add_dep_helper

F32 = mybir.dt.float32
F32R = mybir.dt.float32r
BF16 = mybir.dt.bfloat16

B, C, T, O, D = 64, 273, 2048, 256, 2048
NF = 32
NIJ = NF * NF
NCORES = 8
BLOC = B // NCORES
BC = BLOC * C        # 2184
BCPAD = 2304         # 128*18 padded wrap layout for position prep
MARGIN = 0.2
WIDTH = 1.0 + 2.0 * MARGIN

SEG = 274            # padded per-batch segment width (fp32r needs even N)
BCL = BLOC * SEG     # 2192 padded columns
QWL = 2 * SEG        # 548 (quarter = 2 batches, padded)
PI = math.pi
MAGIC = 12582912.0   # 1.5 * 2^23

_CACHE = {}
LAST_RUN_NS = None

# engine-assignment knobs (tuned against TimelineSim)
SC_ON_ACT = False      # score PSUM->SBUF eviction engine
WT_ON_ACT = False      # transpose PSUM->SBUF (weights) eviction engine
ABS_ON_ACT = True      # |rs| via ACT Abs vs DVE stt max(-rs, rs)
MIX_DVE_MOD = 2        # mix eviction i goes to DVE iff (i % MIX_DVE_MOD) != 0
OUT_DMA_SPLIT = False  # alternate out-DMAs scalar/sync (False: all sync)
OUT_ONE_DMA = True     # one [128,2048] out-DMA per (b,oc) instead of two halves
X_ON_GPSIMD = False    # x-load DMAs dispatched from the idle Pool engine
OUT_ON_GPSIMD = False  # out-DMAs dispatched from the idle Pool engine
PAIR_MIX = False       # mix PSUM tiles span 2 banks (1024-col evictions)
PSUM_F, PSUM_SC, PSUM_TP, PSUM_MIX = 3, 1, 1, 3  # pool bufs
# ics whose kt runs on ACT (Copy bias=M); their rs comes out negated, fixed
# by host-negating the matching sin head rows
KT_ACT_ICS = frozenset()
RS_BUFS, TRIG_BUFS, K_BUFS, WT_BUFS, OEV_BUFS, X_BUFS = 4, 8, 6, 6, 2, 3
STAGGERED_LOOP = False   # For_i staggered semaphore reset (timing loop only)
INTERLEAVE = True        # software-pipeline the emission order


def _consts():
    p = np.arange(NIJ)
    i = (p // NF).astype(np.float32)
    j = (p % NF).astype(np.float32)
    f6 = np.stack([i, i, i, j, j, j]).astype(ml_dtypes.bfloat16)
    ident = np.eye(128, dtype=np.float32)
    ones1 = np.ones((1, 128), dtype=ml_dtypes.bfloat16)
    return f6, ident, ones1


def build(nc=None, loop_n=1):
    nc = nc or bacc.Bacc("TRN2", target_bir_lowering=False, debug=False,
                         enable_partition_id=False)

    x_in = nc.dram_tensor("x", [BLOC, C, T], BF16, kind="ExternalInput")
    u_in = nc.dram_tensor("u", [6, BCPAD], BF16, kind="ExternalInput")
    offs_in = nc.dram_tensor("offs", [1, BCL], BF16, kind="ExternalInput")
    headsT_in = nc.dram_tensor("headsT", [D, O], F32R, kind="ExternalInput")
    out_dram = nc.dram_tensor("out", [BLOC, O, T], BF16, kind="ExternalOutput")

    f6_np, ident_np, ones_np = _consts()
    f6_dram = nc.inline_tensor(f6_np, "f6c")
    ident_dram = nc.inline_tensor(ident_np, "identc")
    ones_dram = nc.inline_tensor(ones_np, "onesc")

    with tile.TileContext(nc) as tc:
        if loop_n > 1:
            with tc.For_i(0, loop_n, staggered_reset=STAGGERED_LOOP):
                _build_tile(tc, x_in, u_in, offs_in, headsT_in, out_dram,
                            f6_dram, ident_dram, ones_dram)
        else:
            _build_tile(tc, x_in, u_in, offs_in, headsT_in, out_dram,
                        f6_dram, ident_dram, ones_dram)
    nc.compile()
    return nc


def _build_tile(tc, x_in, u_in, offs_in, headsT_in, out_dram,
                f6_dram, ident_dram, ones_dram):
    nc = tc.nc
    Sin = mybir.ActivationFunctionType.Sin
    Exp = mybir.ActivationFunctionType.Exp
    Copy = mybir.ActivationFunctionType.Copy
    Abs = mybir.ActivationFunctionType.Abs
    ALU = mybir.AluOpType

    import contextlib
    ctx = contextlib.ExitStack()

    singles = ctx.enter_context(tc.tile_pool(name="singles", bufs=1))
    # U[6, BCPAD]: 3 bf16 limbs of u=(posx+m)/w, 3 of v; host-encoded so
    # the embedding matmuls can start as soon as this one DMA lands.
    # u/f6 ride the scalar-engine HWDGE queue so the first embedding
    # matmuls don't wait behind the 2MB heads DMA on the sync queue.
    u_sb = singles.tile([6, BCPAD], BF16, name="u_sb")
    nc.scalar.dma_start(out=u_sb, in_=u_in.ap())
    f6_sb = singles.tile([6, NIJ], BF16, name="f6_sb")
    nc.scalar.dma_start(out=f6_sb, in_=f6_dram.ap())
    ident_sb = singles.tile([128, 128], F32R, name="ident_sb")
    nc.sync.dma_start(out=ident_sb, in_=ident_dram.ap().bitcast(F32R))
    ones_sb = singles.tile([1, 128], BF16, name="ones_sb")
    nc.sync.dma_start(out=ones_sb, in_=ones_dram.ap())
    offs_sb = singles.tile([1, BCL], BF16, name="offs_sb")
    nc.sync.dma_start(out=offs_sb, in_=offs_in.ap())
    hpi_sb = singles.tile([128, 1], F32, name="hpi_sb")
    nc.vector.memset(hpi_sb, PI / 2)


    # heads, pre-transposed on host: hT[dl, ic*O + o] = headsT[ic*128+dl, o]
    hT = singles.tile([128, 16 * O], F32R, name="hT")
    nc.sync.dma_start(
        out=hT.rearrange("dl (ic o) -> dl ic o", o=O),
        in_=headsT_in.ap().rearrange("(ic dl) o -> dl ic o", dl=128))

    # --- pools ---
    # PSUM budget (8 banks): f + scores + transpose 1 + 2*mix pairs
    f_ps = ctx.enter_context(tc.tile_pool(name="f_ps", bufs=PSUM_F, space="PSUM"))
    sc_ps = ctx.enter_context(tc.tile_pool(name="sc_ps", bufs=PSUM_SC, space="PSUM"))
    tp_ps = ctx.enter_context(tc.tile_pool(name="tp_ps", bufs=PSUM_TP, space="PSUM"))
    mix_ps = ctx.enter_context(tc.tile_pool(name="mix_ps", bufs=PSUM_MIX, space="PSUM"))

    rs_pool = ctx.enter_context(tc.tile_pool(name="rs_pool", bufs=RS_BUFS))
    trig_pool = ctx.enter_context(tc.tile_pool(name="trig_pool", bufs=TRIG_BUFS))
    k_pool = ctx.enter_context(tc.tile_pool(name="k_pool", bufs=K_BUFS))
    sc_sb_pool = ctx.enter_context(tc.tile_pool(name="sc_sb", bufs=1))
    sums_pool = ctx.enter_context(tc.tile_pool(name="sums", bufs=1))
    wt_pool = ctx.enter_context(tc.tile_pool(name="wt", bufs=WT_BUFS))
    x_pool = ctx.enter_context(tc.tile_pool(name="x_pool", bufs=X_BUFS))
    oev_pool = ctx.enter_context(tc.tile_pool(name="oev", bufs=OEV_BUFS))

    SC = [sc_sb_pool.tile([128, BCL], F32R, name=f"SC{oc}") for oc in range(2)]
    sums = sums_pool.tile([128, 2 * BLOC], F32, name="sums")
    rsums = sums_pool.tile([128, 2 * BLOC], F32, name="rsums")

    CW = [(0, 128), (128, 128), (256, C - 256)]

    sin_insts_group = [[], []]
    exp_insts_group = [[], []]
    trig_list = {}
    mix_evict_ctr = [0]

    def embed_pc(q, pc):
        g = q // 2
        # rs tile: [rs block | abs block], each 2*QWL
        rs_t = rs_pool.tile([128, 4 * QWL], F32, tag="rs")
        for half in range(2):
            ic = 2 * pc + half
            kt_on_act = ic in KT_ACT_ICS
            for bi in range(2):
                ucol = q * QWL + bi * SEG
                fp = f_ps.tile([128, SEG], F32, tag="f", name="fp")
                nc.tensor.matmul(
                    fp,
                    f6_sb[:, ic * 128:(ic + 1) * 128],
                    u_sb[:, ucol:ucol + SEG],
                    start=True, stop=True)
                col = half * QWL + bi * SEG
                kt = k_pool.tile([128, SEG], F32, tag="kt", name="kt")
                if kt_on_act:
                    # kt_pre = RN(f + M) = M + round(f) on ACT (the fp32
                    # output write rounds); rs' = -rs, absorbed by
                    # host-negating this ic's sin head rows
                    nc.scalar.activation(kt, fp, Copy, bias=MAGIC)
                    nc.vector.scalar_tensor_tensor(
                        rs_t[:, col:col + SEG], kt, MAGIC, fp,
                        ALU.subtract, ALU.subtract)
                else:
                    # kt = round(f): fp32 rounding of (f + M) - M is
                    # exact round-to-nearest for |f| << 2^22
                    nc.vector.tensor_scalar(
                        kt, fp, MAGIC, MAGIC, ALU.add, ALU.subtract)
                    # rs = f - round(f)  in [-0.5, 0.5], exact
                    nc.vector.scalar_tensor_tensor(
                        rs_t[:, col:col + SEG], fp, 0.0, kt,
                        ALU.add, ALU.subtract)
        # |rs| for the cos fold (Sin table is only valid to ~1.26pi)
        if ABS_ON_ACT:
            nc.scalar.activation(rs_t[:, 2 * QWL:], rs_t[:, :2 * QWL], Abs)
        else:
            nc.vector.scalar_tensor_tensor(
                rs_t[:, 2 * QWL:], rs_t[:, :2 * QWL], -1.0,
                rs_t[:, :2 * QWL], ALU.mult, ALU.max)
        # separate F32R tile: the fp32r score matmuls require their operand
        # bytes to be produced f32r-rounded (BIR verifier checks every
        # overlapping writer, so in-place is out)
        trig_t = trig_pool.tile([128, 4 * QWL], F32R, tag="trig",
                                name=f"trig_q{q}p{pc}")
        si = nc.scalar.activation(trig_t[:, :2 * QWL],
                                  rs_t[:, :2 * QWL], Sin,
                                  bias=0.0, scale=2 * PI)
        ci = nc.scalar.activation(trig_t[:, 2 * QWL:],
                                  rs_t[:, 2 * QWL:], Sin,
                                  bias=hpi_sb, scale=-2 * PI)
        sin_insts_group[g] += [si, ci]
        if g == 1:
            for e in exp_insts_group[0]:
                add_dep_helper(si.ins, e.ins, sync=False,
                               reason="ACT order: g1 sins after g0 exps")
                add_dep_helper(ci.ins, e.ins, sync=False,
                               reason="ACT order: g1 sins after g0 exps")
        trig_list[(q, pc)] = trig_t

    def scores_quarter(q, embeds=()):
        """Scores for quarter q, pc-major: each oc-pair of tiles starts as
        soon as trig(q, pc0) lands instead of waiting for the whole trig
        chain.  `embeds` are deferred embed_pc thunks for a later quarter,
        interleaved into the emission stream so the in-order PE sequencer
        can issue their f-matmuls between trig-gated score chunks."""
        embeds = list(embeds)
        for oc in range(2):
            sps = {}
            for bi in range(2):
                sps[bi] = sc_ps.tile([128, SEG], F32, tag="sc",
                                     name=f"sp{oc}{bi}")
            for pc in range(4):
                trig_t = trig_list[(q, pc)]
                for bi in range(2):
                    sp = sps[bi]
                    for half in range(2):
                        ic = 2 * pc + half
                        col = half * QWL + bi * SEG
                        # cos block (cols 2*QWL..), then sin block
                        nc.tensor.matmul(
                            sp,
                            hT[:, ic * O + oc * 128: ic * O + oc * 128 + 128],
                            trig_t[:, 2 * QWL + col:2 * QWL + col + SEG],
                            start=(pc == 0 and half == 0), stop=False,
                            skip_group_check=True)
                        nc.tensor.matmul(
                            sp,
                            hT[:, (8 + ic) * O + oc * 128: (8 + ic) * O + oc * 128 + 128],
                            trig_t[:, col:col + SEG],
                            start=False, stop=False, skip_group_check=True)
                if embeds:
                    embeds.pop(0)()
            for bi in range(2):
                b = q * 2 + bi
                sp = sps[bi]
                nc.tensor.matmul(
                    sp, ones_sb,
                    offs_sb[:, b * SEG:b * SEG + SEG],
                    start=False, stop=True, skip_group_check=True)
                if SC_ON_ACT:
                    nc.scalar.activation(
                        SC[oc][:, b * SEG:b * SEG + SEG],
                        sp.bitcast(F32R), Copy)
                else:
                    nc.vector.tensor_copy(
                        SC[oc][:, b * SEG:b * SEG + SEG],
                        sp.bitcast(F32R))
        while embeds:
            embeds.pop(0)()

    def mix_batch(b):
            g = b // 4
            seg = slice(b * SEG, b * SEG + C)
            for oc in range(2):
                ei = nc.scalar.activation(
                    SC[oc][:, seg], SC[oc][:, seg], Exp,
                    accum_out=sums[:, 2 * b + oc: 2 * b + oc + 1])
                for s in sin_insts_group[g]:
                    add_dep_helper(ei.ins, s.ins, sync=False,
                                   reason="ACT order: exps after group sins")
                if g == 0:
                    # interleaved emission: some g1 sins may already exist;
                    # keep them ordered after every g0 exp
                    for s in sin_insts_group[1]:
                        add_dep_helper(s.ins, ei.ins, sync=False,
                                       reason="ACT order: g1 sins after g0 exps")
                exp_insts_group[g].append(ei)
            # both oc sums are adjacent: one reciprocal per batch
            nc.vector.reciprocal(
                rsums[:, 2 * b: 2 * b + 2],
                sums[:, 2 * b: 2 * b + 2])

            wts = []
            for kc, (c0, cw) in enumerate(CW):
                wt = wt_pool.tile([128, O], BF16, tag=f"wt{kc}")
                wts.append(wt)
                tp = tp_ps.tile([128, O], F32R, tag="tp", name="tp")
                for oc in range(2):
                    nc.tensor.transpose(
                        tp[:cw, oc * 128:(oc + 1) * 128],
                        SC[oc][:, b * SEG + c0: b * SEG + c0 + cw],
                        ident_sb)
                if WT_ON_ACT:
                    nc.scalar.activation(wt[:cw, :], tp[:cw, :], Copy)
                else:
                    nc.vector.tensor_copy(wt[:cw, :], tp[:cw, :])

            xts = []
            x_eng = nc.gpsimd if X_ON_GPSIMD else nc.sync
            for kc, (c0, cw) in enumerate(CW):
                xt = x_pool.tile([128, T], BF16, tag=f"x{kc}")
                xts.append(xt)
                x_eng.dma_start(out=xt[:cw, :], in_=x_in.ap()[b, c0:c0 + cw, :])

            def _mix_evict(oslice, op, rsum_col):
                i = mix_evict_ctr[0]
                mix_evict_ctr[0] += 1
                # fused softmax-normalization + bf16 cast eviction,
                # spread DVE/ACT (Pool cannot read PSUM)
                if i % MIX_DVE_MOD == 0:
                    nc.scalar.activation(oslice, op, Copy,
                                         bias=0.0, scale=rsum_col)
                else:
                    nc.vector.tensor_scalar(oslice, op, rsum_col,
                                            None, ALU.mult)

            for oc in range(2):
                oe = oev_pool.tile([128, T], BF16, tag="oe")
                rsum_col = rsums[:, 2 * b + oc: 2 * b + oc + 1]
                if PAIR_MIX:
                    for tp2 in range(2):
                        # two-bank PSUM tile: both tt halves land here so
                        # the eviction runs as a single 1024-col op
                        op = mix_ps.tile([128, 1024], F32, tag="mo", name="mop")
                        for tth in range(2):
                            tt = tp2 * 2 + tth
                            for kc, (c0, cw) in enumerate(CW):
                                nc.tensor.matmul(
                                    op[:, tth * 512:(tth + 1) * 512],
                                    wts[kc][:cw, oc * 128:(oc + 1) * 128],
                                    xts[kc][:cw, tt * 512:(tt + 1) * 512],
                                    start=(kc == 0), stop=(kc == 2),
                                    skip_group_check=True)
                        _mix_evict(oe[:, tp2 * 1024:(tp2 + 1) * 1024],
                                   op, rsum_col)
                else:
                    for tt in range(4):
                        op = mix_ps.tile([128, 512], F32, tag="mo", name="mop")
                        for kc, (c0, cw) in enumerate(CW):
                            nc.tensor.matmul(
                                op,
                                wts[kc][:cw, oc * 128:(oc + 1) * 128],
                                xts[kc][:cw, tt * 512:(tt + 1) * 512],
                                start=(kc == 0), stop=(kc == 2),
                                skip_group_check=True)
                        _mix_evict(oe[:, tt * 512:(tt + 1) * 512],
                                   op, rsum_col)
                # two half-DMAs so the first overlaps the tail evictions
                out_eng = (nc.gpsimd if OUT_ON_GPSIMD
                           else nc.scalar if (OUT_DMA_SPLIT and b % 2 == 0)
                           else nc.sync)
                if OUT_ONE_DMA:
                    out_eng.dma_start(
                        out=out_dram.ap()[b, oc * 128:(oc + 1) * 128, :],
                        in_=oe)
                else:
                    for hh in range(2):
                        out_eng.dma_start(
                            out=out_dram.ap()[b, oc * 128:(oc + 1) * 128,
                                              hh * 1024:(hh + 1) * 1024],
                            in_=oe[:, hh * 1024:(hh + 1) * 1024])

    # ---------- orchestration (software-pipelined emission) ----------
    if INTERLEAVE:
        def embed_thunks(q):
            return [lambda q=q, pc=pc: embed_pc(q, pc) for pc in range(4)]
        for pc in range(4):
            embed_pc(0, pc)
        scores_quarter(0, embed_thunks(1))     # scores(q0) x embed(q1)
        scores_quarter(1)
        for bi in range(4):                    # mix(g0) x embed(q2): q2's
            mix_batch(bi)                      # sins auto-order after g0 exps
            embed_pc(2, bi)
        scores_quarter(2, embed_thunks(3))     # scores(q2) x embed(q3)
        scores_quarter(3)
        for bi in range(4):
            mix_batch(4 + bi)
    else:
        for g in range(2):
            for qq in range(2):
                q = g * 2 + qq
                for pc in range(4):
                    embed_pc(q, pc)
                scores_quarter(q)
            for bi in range(4):
                mix_batch(g * 4 + bi)

    ctx.close()


# --------------------------------------------------------------------------
# host side
# --------------------------------------------------------------------------

def _host_inputs(x, positions, invalid_mask, heads):
    headsT = np.ascontiguousarray(np.asarray(heads, dtype=np.float32).T)
    if KT_ACT_ICS:
        headsT = headsT.copy()
        for ic in KT_ACT_ICS:
            # these ics' rs is negated on device (kt-on-ACT path); absorb
            # the sign into the sin head rows (cos is even, unaffected)
            headsT[NIJ + ic * 128: NIJ + (ic + 1) * 128, :] *= -1.0
    x_bf = np.asarray(x, dtype=np.float32).astype(ml_dtypes.bfloat16)
    in_maps = []
    for core in range(NCORES):
        bsl = slice(core * BLOC, (core + 1) * BLOC)
        xs = np.ascontiguousarray(x_bf[bsl])

        pos = positions[bsl].reshape(BLOC, C, 2).astype(np.float64)
        uvw = np.zeros((2, BCPAD), dtype=np.float64)
        for ax in range(2):
            seg = uvw[ax, :BCL].reshape(BLOC, SEG)
            seg[:, :C] = (pos[:, :, ax] + MARGIN) / WIDTH
        u6 = np.zeros((6, BCPAD), dtype=ml_dtypes.bfloat16)
        for ax in range(2):
            resid = uvw[ax].copy()
            for li in range(3):
                limb = resid.astype(ml_dtypes.bfloat16)
                u6[3 * ax + li] = limb
                resid = resid - limb.astype(np.float64)

        offs = np.zeros((1, BCL), dtype=np.float32)
        offs[0, :].reshape(BLOC, SEG)[:, :C] = np.where(
            invalid_mask[bsl], -1e30, 0.0)
        offs = offs.astype(ml_dtypes.bfloat16)

        in_maps.append({"x": xs, "u": u6, "offs": offs, "headsT": headsT})
    return in_maps


def kernel(**inputs):
    global LAST_RUN_NS
    from concourse.bass_utils import run_bass_kernel_spmd

    x = np.asarray(inputs["x"])
    positions = np.asarray(inputs["positions"])
    invalid_mask = np.asarray(inputs["invalid_mask"])
    heads = np.asarray(inputs["heads"])

    if "nc" not in _CACHE:
        _CACHE["nc"] = build()
    nc = _CACHE["nc"]

    in_maps = _host_inputs(x, positions, invalid_mask, heads)
    t0 = time.perf_counter()
    res = run_bass_kernel_spmd(nc, in_maps, core_ids=list(range(NCORES)))
    LAST_RUN_NS = (time.perf_counter() - t0) * 1e9
    out = np.concatenate([r["out"] for r in res.results], axis=0)
    return out.astype(np.float32)
